# revision 1
# baseline (speedup 1.0000x reference)
"""Trainium2 Bass kernel for nn_CatEncoderCross.

Computes out[b,i,j,:] = input1[b,i,:] @ W[:768] + input2[b,j,:] @ W[768:] + bias
for shapes input1/input2 [4,128,768], W [1536,768], b [768],
output [4,128,128,768] (f32, ~192 MB).

Sharding: (batch, dout-half): core c handles batch c//2 and output columns
[384*(c%2), 384*(c%2)+384).  Halves the per-core W read (2.36 MB vs 4.72)
while keeping output DMA descriptors at 1536 B (>=512 B line-rate).  Per core:
  p1 = x1 @ W1[:, half]          [128, 384]  (PE, bf16 hi/lo x 3 terms)
  p2 = x2 @ W2[:, half] + bias   [128, 384]  (PE)
  for each i: psum = ones x p1[i]  (K=2 hi/lo-stacked broadcast matmul)
              out_tile = psum + p2 (DVE tensor_tensor, 4 tiles fused)
              DMA 4 tiles -> HBM
"""

import os
import numpy as np

P = 128
DC = 768  # contraction depth per W half (d1 = d2)
DO = 384  # output columns per core (dout/2)
KO = 6  # K chunks of 128 in d1 (=d2)
NI = 128  # n1 rows per core (full batch)
NJ = 128  # n2
NCORES = 8
FUSE = 4  # output tiles per psum group / DVE op / DMA

OUT_BUFS = int(os.environ.get("KERNEL_OUT_BUFS", "4"))
PSUM_BUFS = int(os.environ.get("KERNEL_PSUM_BUFS", "2"))
WARM_MMS = int(os.environ.get("KERNEL_WARM_MMS", "30"))
# perf-probe only: comma list of stages to skip (never set in real runs)
SKIP = set(s for s in os.environ.get("KERNEL_SKIP", "").split(",") if s)

_cache = {}


def _split_hl(x):
    """Split fp32 array into bf16 hi + lo with x ~= hi + lo."""
    import ml_dtypes

    hi = x.astype(ml_dtypes.bfloat16)
    lo = (x - hi.astype(np.float32)).astype(ml_dtypes.bfloat16)
    return hi, lo


def _build_module():
    import concourse.bacc as bacc
    import concourse.mybir as mybir
    import concourse.tile as tile

    F32 = mybir.dt.float32
    BF16 = mybir.dt.bfloat16

    nc = bacc.Bacc("TRN2", target_bir_lowering=False, debug=False)

    # --- DRAM I/O (hi/lo planes interleaved per k-chunk for W) ---
    x1T_d = nc.dram_tensor("x1T", [P, 2 * KO, NI], BF16, kind="ExternalInput")
    x2T_d = nc.dram_tensor("x2T", [P, 2 * KO, NJ], BF16, kind="ExternalInput")
    w_d = nc.dram_tensor("Wr", [P, 4 * KO, DO], BF16, kind="ExternalInput")
    bias_d = nc.dram_tensor("biasr", [2, DO], BF16, kind="ExternalInput")
    out_d = nc.dram_tensor("out", [NI, NJ, DO], F32, kind="ExternalOutput")
    out_ap = out_d.ap()

    with tile.TileContext(nc) as tc:
        with (
            tc.tile_pool(name="const", bufs=1) as cpool,
            tc.tile_pool(name="psum", bufs=PSUM_BUFS, space="PSUM") as pspool,
            tc.tile_pool(name="outp", bufs=OUT_BUFS) as opool,
        ):
            w1_grp = [
                cpool.tile([P, 2, 2, DO], BF16, tag=f"wg{g}", name=f"wg{g}")
                for g in range(KO // 2)
            ]
            w_pair = [w1_grp[o // 2][:, o % 2] for o in range(KO)] + [
                cpool.tile([P, 2, DO], BF16, tag=f"w{o}", name=f"w{o}")
                for o in range(KO, 2 * KO)
            ]
            w_sb = [w_pair[o // 2][:, o % 2] for o in range(4 * KO)]
            x1T_sb = cpool.tile([P, 2 * KO, NI], BF16, tag="x1T")
            x2T_sb = cpool.tile([P, 2 * KO, NJ], BF16, tag="x2T")
            bias_sb = cpool.tile([2, DO], BF16, tag="bias")
            ones_bf = cpool.tile([98, P], BF16, tag="ones_bf")
            p1_sb = cpool.tile([NI, DO], F32, tag="p1")
            p2_sb = cpool.tile([P, DO], F32, tag="p2")
            p1h_sb = cpool.tile([NI, DO], BF16, tag="p1h")
            p1l_sb = cpool.tile([NI, DO], BF16, tag="p1l")
            # p1 rows flattened: group g rows [32g,32g+32) -> partitions
            # 32g (hi plane) and 32g+1 (lo plane)
            p1f = cpool.tile([98, 32, DO], BF16, tag="p1f")

            nc.vector.memset(ones_bf[:], 1.0)

            if WARM_MMS:
                warm_ps = pspool.tile([P, 2048], F32, tag="ps", name="warm_ps")
                for wi in range(WARM_MMS):
                    nc.tensor.matmul(
                        warm_ps[:, 0:P],
                        ones_bf[0:2, :],
                        ones_bf[0:2, :],
                        start=True,
                        stop=True,
                    )

            # --- input DMAs: x1T, W1 chunks, x2T, bias, W2 chunks (the
            # last W2 chunk split hi/lo so one MM trails the last byte) ---
            wr = w_d.ap().rearrange("p (o x) d -> p o x d", x=2)
            wr2 = w_d.ap().rearrange("p (g o x) d -> p g o x d", x=2, o=2)
            nc.sync.dma_start(out=x1T_sb[:], in_=x1T_d.ap())
            for g in range(KO // 2):
                nc.sync.dma_start(out=w1_grp[g][:], in_=wr2[:, g])
            nc.sync.dma_start(out=x2T_sb[:], in_=x2T_d.ap())
            nc.sync.dma_start(out=bias_sb[:], in_=bias_d.ap())
            for o in range(KO, 2 * KO):
                if o == 2 * KO - 1:
                    nc.sync.dma_start(out=w_pair[o][:, 0], in_=wr[:, o, 0])
                    nc.sync.dma_start(out=w_pair[o][:, 1], in_=wr[:, o, 1])
                else:
                    nc.sync.dma_start(out=w_pair[o][:], in_=wr[:, o])

            def proj_matmuls(ps, x_sb, w_off, head):
                # per k-chunk: xh@Wh, xl@Wh, xh@Wl (x planes [0:KO]=hi,
                # [KO:2KO]=lo; W planes interleaved hi,lo per chunk)
                steps = []
                for o in range(KO):
                    steps.append((o, 2 * (w_off + o) + 0))
                    steps.append((KO + o, 2 * (w_off + o) + 0))
                    steps.append((o, 2 * (w_off + o) + 1))
                for si, (xo, wo) in enumerate(steps):
                    nc.tensor.matmul(
                        ps[:, 0:DO],
                        x_sb[:, xo, :],
                        w_sb[wo][:],
                        start=(si == 0 and not head),
                        stop=(si == len(steps) - 1),
                    )

            # --- p1 = x1 @ W1h ---
            do_proj = "proj" not in SKIP
            if do_proj:
                p1_ps = pspool.tile([P, 2048], F32, tag="ps", name="p1_ps")
                proj_matmuls(p1_ps, x1T_sb, 0, head=False)
                nc.vector.tensor_copy(out=p1_sb[:], in_=p1_ps[:, 0:DO])

                # --- p2 = x2 @ W2h + bias (bias first: it lands early) ---
                p2_ps = pspool.tile([P, 2048], F32, tag="ps", name="p2_ps")
                nc.tensor.matmul(
                    p2_ps[:, 0:DO],
                    ones_bf[0:2, :],
                    bias_sb[:],
                    start=True,
                    stop=False,
                )
                proj_matmuls(p2_ps, x2T_sb, KO, head=True)
                nc.scalar.mul(p2_sb[:], p2_ps[:, 0:DO], 1.0)

                # --- p1 hi/lo split + flatten to broadcast layout ---
                nc.vector.tensor_copy(out=p1h_sb[:], in_=p1_sb[:])
                nc.vector.tensor_sub(out=p1l_sb[:], in0=p1_sb[:], in1=p1h_sb[:])
                for g in range(4):
                    nc.sync.dma_start(
                        out=p1f[32 * g : 32 * g + 1],
                        in_=p1h_sb[32 * g : 32 * g + 32, :],
                    )
                    nc.sync.dma_start(
                        out=p1f[32 * g + 1 : 32 * g + 2],
                        in_=p1l_sb[32 * g : 32 * g + 32, :],
                    )

            # --- main loop: 32 groups of 4 tiles ---
            for grp in range(NI // FUSE):
                i0 = FUSE * grp
                ob = opool.tile([P, FUSE, DO], F32, tag="ob", name=f"ob{grp}")
                ps = pspool.tile([P, 2048], F32, tag="ps", name=f"ps{grp}")
                if "mm1" not in SKIP:
                    for m in range(FUSE):
                        i = i0 + m
                        g = i // 32
                        lhsT = ones_bf[32 * g : 32 * g + 2, :]
                        rhs = p1f[32 * g : 32 * g + 2, i % 32, :]
                        nc.tensor.matmul(
                            ps[:, 512 * m : 512 * m + DO],
                            lhsT,
                            rhs,
                            start=True,
                            stop=True,
                            tile_position=(32 * g, 0),
                        )
                ps_v = ps.rearrange("p (i x) -> p i x", i=FUSE)[:, :, 0:DO]
                if grp == 0 and "add" not in SKIP and "dmaout" not in SKIP:
                    # unfused first tile: the write stream starts after a
                    # single [128,384] add instead of a 4-wide fused one
                    nc.vector.tensor_add(
                        out=ob[:, 0, :], in0=ps_v[:, 0, :], in1=p2_sb[:]
                    )
                    nc.sync.dma_start(
                        out=out_ap[0:1].rearrange("i j d -> j i d"),
                        in_=ob[:, 0:1, :],
                    )
                    p2_b3 = p2_sb[:, None, :].to_broadcast((P, FUSE - 1, DO))
                    nc.vector.tensor_add(
                        out=ob[:, 1:FUSE, :], in0=ps_v[:, 1:FUSE, :], in1=p2_b3
                    )
                    nc.sync.dma_start(
                        out=out_ap[1:FUSE].rearrange("i j d -> j i d"),
                        in_=ob[:, 1:FUSE, :],
                    )
                    continue
                if "add" not in SKIP:
                    p2_b = p2_sb[:, None, :].to_broadcast((P, FUSE, DO))
                    nc.vector.tensor_add(out=ob[:], in0=ps_v, in1=p2_b)
                if "dmaout" not in SKIP:
                    dst = out_ap[i0 : i0 + FUSE]  # [FUSE, NJ, DO]
                    nc.sync.dma_start(
                        out=dst.rearrange("i j d -> j i d"), in_=ob[:]
                    )

    nc.compile()
    return nc


def _get_module():
    key = (OUT_BUFS, PSUM_BUFS, WARM_MMS, tuple(sorted(SKIP)))
    if key not in _cache:
        _cache[key] = _build_module()
    return _cache[key]


def _prep_xT(x, n):
    """[rows, 768] -> [128, KO, rows] transposed chunk layout."""
    return np.ascontiguousarray(x.T.reshape(KO, P, n).transpose(1, 0, 2))


def _make_in_maps(input1, input2, W, b):
    input1 = np.asarray(input1, dtype=np.float32)
    input2 = np.asarray(input2, dtype=np.float32)
    W = np.asarray(W, dtype=np.float32)
    b = np.asarray(b, dtype=np.float32)

    in_maps = []
    for c in range(NCORES):
        bb, h = divmod(c, 2)
        Whalf = np.ascontiguousarray(W[:, h * DO : (h + 1) * DO])
        Wh, Wl = _split_hl(Whalf)
        Wrh = Wh.reshape(2 * KO, P, DO).transpose(1, 0, 2)
        Wrl = Wl.reshape(2 * KO, P, DO).transpose(1, 0, 2)
        Wr = np.ascontiguousarray(
            np.stack([Wrh, Wrl], axis=2).reshape(P, 4 * KO, DO)
        )
        bh, bl = _split_hl(b[h * DO : (h + 1) * DO])
        biasr = np.ascontiguousarray(np.stack([bh, bl], axis=0))

        x1h, x1l = _split_hl(input1[bb])
        x2h, x2l = _split_hl(input2[bb])
        x1T = np.ascontiguousarray(
            np.concatenate([_prep_xT(x1h, NI), _prep_xT(x1l, NI)], axis=1)
        )
        x2T = np.ascontiguousarray(
            np.concatenate([_prep_xT(x2h, NJ), _prep_xT(x2l, NJ)], axis=1)
        )
        in_maps.append({"x1T": x1T, "x2T": x2T, "Wr": Wr, "biasr": biasr})
    return in_maps


def kernel(input1, input2, W, b):
    from concourse import bass_utils

    suppress_trace = False
    if os.environ.get("BASS_TRACE"):
        try:
            from antenv.axon_hooks import get_axon_ntff_profile_hook  # noqa: F401
        except Exception:
            suppress_trace = True
    prev = os.environ.get("BASS_NEVER_TRACE")
    if suppress_trace:
        os.environ["BASS_NEVER_TRACE"] = "1"
    try:
        nc = _get_module()
        in_maps = _make_in_maps(input1, input2, W, b)
        res = bass_utils.run_bass_kernel_spmd(
            nc, in_maps, core_ids=list(range(NCORES))
        )
    finally:
        if suppress_trace:
            if prev is None:
                os.environ.pop("BASS_NEVER_TRACE", None)
            else:
                os.environ["BASS_NEVER_TRACE"] = prev
    out = np.empty((4, NJ, NJ, 2 * DO), dtype=np.float32)
    for c in range(NCORES):
        bb, h = divmod(c, 2)
        out[bb, :, :, h * DO : (h + 1) * DO] = res.results[c]["out"]
    return out



# revision 4
# speedup vs baseline: 1.2374x; 1.2374x over previous
"""Trainium2 Bass kernel for nn_CatEncoderCross.

Computes out[b,i,j,:] = input1[b,i,:] @ W[:768] + input2[b,j,:] @ W[768:] + bias
for shapes input1/input2 [4,128,768], W [1536,768], b [768],
output [4,128,128,768] f32 (~201 MB).

Sharding (batch, dout-half): core c handles batch c//2 and output columns
[384*(c%2), 384*(c%2)+384).

Per core, with j on partitions and 4 i-rows per group (one PSUM bank each):
  p1 = x1 @ W1[:, half]          [128, 384]  (PE, single-plane bf16)
  p2 = x2 @ W2[:, half] + bias   [128, 384]  (PE)
  group g (rows i0..i0+3):
    banks 0,1: psum = ones x p1[i]          -> DVE / Pool add p2, write f16
    banks 2,3: psum = eye @ p2 + ones x p1[i] -> Act copies psum to f16
  DMA [128, 4, 384] f16 tile -> HBM.

The identity-matmul ("eye") accumulates p2 into PSUM on the PE so the
Activation engine (which can only add per-partition scalars, not vectors)
can participate as a pure psum->f16 converter.  Output is written fp16
(host upcasts); inputs are single-plane bf16.  Measured rel err ~2.5e-3
vs the f32 reference (gate 2e-2).
"""

import os
import numpy as np

P = 128
DO = 384  # output columns per core (dout/2)
KO = 6  # K chunks of 128 in d1 (=d2)
NI = 128  # n1 rows per core (full batch)
NJ = 128  # n2
NCORES = 8
FUSE = 4  # output rows per psum group / DMA

OUT_BUFS = int(os.environ.get("KERNEL_OUT_BUFS", "4"))
PSUM_BUFS = int(os.environ.get("KERNEL_PSUM_BUFS", "2"))
WARM_MMS = int(os.environ.get("KERNEL_WARM_MMS", "30"))
# perf-probe only: comma list of stages to skip (never set in real runs)
SKIP = set(s for s in os.environ.get("KERNEL_SKIP", "").split(",") if s)

_cache = {}


def _build_module():
    import concourse.bacc as bacc
    import concourse.mybir as mybir
    import concourse.tile as tile

    F32 = mybir.dt.float32
    F16 = mybir.dt.float16
    BF16 = mybir.dt.bfloat16

    nc = bacc.Bacc("TRN2", target_bir_lowering=False, debug=False)

    x1T_d = nc.dram_tensor("x1T", [P, KO, NI], BF16, kind="ExternalInput")
    x2T_d = nc.dram_tensor("x2T", [P, KO, NJ], BF16, kind="ExternalInput")
    w1_d = nc.dram_tensor("w1r", [P, KO, DO], BF16, kind="ExternalInput")
    w2_d = nc.dram_tensor("w2r", [P, KO, DO], BF16, kind="ExternalInput")
    eye_d = nc.dram_tensor("eyer", [P, P], BF16, kind="ExternalInput")
    bias_d = nc.dram_tensor("biasr", [2, DO], BF16, kind="ExternalInput")
    out_d = nc.dram_tensor("out", [NI, NJ, DO], F16, kind="ExternalOutput")
    out_ap = out_d.ap()

    with tile.TileContext(nc) as tc:
        with (
            tc.tile_pool(name="const", bufs=1) as cpool,
            tc.tile_pool(name="psum", bufs=PSUM_BUFS, space="PSUM") as pspool,
            tc.tile_pool(name="outp", bufs=OUT_BUFS) as opool,
        ):
            x1T_sb = cpool.tile([P, KO, NI], BF16, tag="x1T")
            x2T_sb = cpool.tile([P, KO, NJ], BF16, tag="x2T")
            w1_sb = cpool.tile([P, KO, DO], BF16, tag="w1")
            w2_sb = cpool.tile([P, KO, DO], BF16, tag="w2")
            eye_sb = cpool.tile([P, P], BF16, tag="eye")
            bias_sb = cpool.tile([2, DO], BF16, tag="bias")
            ones_bf = cpool.tile([2, P], BF16, tag="ones_bf")
            p1h_sb = cpool.tile([NI, DO], BF16, tag="p1h")
            # all 128 p1 rows flattened into partition 0 for the ones-matmul
            # broadcast (rhs = p1f[0:1, i, :])
            p1f = cpool.tile([1, NI, DO], BF16, tag="p1f")
            p2_sb = cpool.tile([P, DO], F32, tag="p2")
            p2bf_sb = cpool.tile([P, DO], BF16, tag="p2bf")

            nc.vector.memset(ones_bf[:], 1.0)

            if WARM_MMS:
                warm_ps = pspool.tile([P, 2048], F32, tag="ps", name="warm_ps")
                for wi in range(WARM_MMS):
                    nc.tensor.matmul(
                        warm_ps[:, 0:P],
                        ones_bf[0:2, :],
                        ones_bf[0:2, :],
                        start=True,
                        stop=True,
                    )

            # --- input DMAs (issue order = SP queue order) ---
            nc.sync.dma_start(out=x1T_sb[:], in_=x1T_d.ap())
            nc.sync.dma_start(out=w1_sb[:], in_=w1_d.ap())
            nc.sync.dma_start(out=x2T_sb[:], in_=x2T_d.ap())
            for o in range(KO):
                nc.sync.dma_start(out=w2_sb[:, o], in_=w2_d.ap()[:, o])
            nc.sync.dma_start(out=eye_sb[:], in_=eye_d.ap())
            nc.sync.dma_start(out=bias_sb[:], in_=bias_d.ap())

            do_proj = "proj" not in SKIP
            if do_proj:
                # --- p1 = x1 @ W1h ---
                p1_ps = pspool.tile([P, 2048], F32, tag="ps", name="p1_ps")
                for o in range(KO):
                    nc.tensor.matmul(
                        p1_ps[:, 0:DO],
                        x1T_sb[:, o, :],
                        w1_sb[:, o, :],
                        start=(o == 0),
                        stop=(o == KO - 1),
                    )
                nc.vector.tensor_copy(out=p1h_sb[:], in_=p1_ps[:, 0:DO])
                # flatten p1 rows into partition 0 (one DMA, 128 descriptors)
                nc.sync.dma_start(out=p1f[0:1], in_=p1h_sb[:])

                # --- p2 = x2 @ W2h + bias ---
                p2_ps = pspool.tile([P, 2048], F32, tag="ps", name="p2_ps")
                nc.tensor.matmul(
                    p2_ps[:, 0:DO],
                    ones_bf[0:2, :],
                    bias_sb[:],
                    start=True,
                    stop=False,
                )
                for o in range(KO):
                    nc.tensor.matmul(
                        p2_ps[:, 0:DO],
                        x2T_sb[:, o, :],
                        w2_sb[:, o, :],
                        start=False,
                        stop=(o == KO - 1),
                    )
                nc.scalar.mul(p2_sb[:], p2_ps[:, 0:DO], 1.0)
                nc.vector.tensor_copy(out=p2bf_sb[:], in_=p2_ps[:, 0:DO])

            # --- main loop: 32 groups of 4 rows ---
            for grp in range(NI // FUSE):
                i0 = FUSE * grp
                ob = opool.tile([P, FUSE, DO], F16, tag="ob", name=f"ob{grp}")
                ps = pspool.tile([P, 2048], F32, tag="ps", name=f"ps{grp}")
                if "mm1" not in SKIP:
                    for m in range(FUSE):
                        i = i0 + m
                        dst = ps[:, 512 * m : 512 * m + DO]
                        if m >= 2:
                            # eye-bank: accumulate p2 via identity matmul
                            nc.tensor.matmul(
                                dst,
                                eye_sb[:],
                                p2bf_sb[:],
                                start=True,
                                stop=False,
                            )
                        nc.tensor.matmul(
                            dst,
                            ones_bf[0:1, :],
                            p1f[0:1, i, :],
                            start=(m < 2),
                            stop=True,
                        )
                ps_v = ps.rearrange("p (i x) -> p i x", i=FUSE)[:, :, 0:DO]
                if "add" not in SKIP:
                    p2_b = p2_sb[:, None, :].to_broadcast((P, 2, DO))
                    nc.vector.tensor_add(
                        out=ob[:, 0:2, :], in0=ps_v[:, 0:2, :], in1=p2_b
                    )
                    nc.scalar.copy(out=ob[:, 2:4, :], in_=ps_v[:, 2:4, :])
                if "dmaout" not in SKIP:
                    dst = out_ap[i0 : i0 + FUSE]  # [FUSE, NJ, DO]
                    nc.sync.dma_start(
                        out=dst.rearrange("i j d -> j i d"), in_=ob[:]
                    )

    nc.compile()
    return nc


def _get_module():
    key = (OUT_BUFS, PSUM_BUFS, WARM_MMS, tuple(sorted(SKIP)))
    if key not in _cache:
        _cache[key] = _build_module()
    return _cache[key]


def _bf16(x):
    import ml_dtypes

    return np.asarray(x, dtype=np.float32).astype(ml_dtypes.bfloat16)


def _prep_xT(x):
    """[128, 768] -> [128, KO, 128] transposed chunk layout (bf16)."""
    return np.ascontiguousarray(x.T.reshape(KO, P, P).transpose(1, 0, 2))


def _make_in_maps(input1, input2, W, b):
    import ml_dtypes

    input1 = np.asarray(input1, dtype=np.float32)
    input2 = np.asarray(input2, dtype=np.float32)
    W = np.asarray(W, dtype=np.float32)
    b = np.asarray(b, dtype=np.float32)

    eye = np.eye(P, dtype=ml_dtypes.bfloat16)
    in_maps = []
    for c in range(NCORES):
        bb, h = divmod(c, 2)
        W1 = _bf16(W[:768, h * DO : (h + 1) * DO])
        W2 = _bf16(W[768:, h * DO : (h + 1) * DO])
        w1r = np.ascontiguousarray(W1.reshape(KO, P, DO).transpose(1, 0, 2))
        w2r = np.ascontiguousarray(W2.reshape(KO, P, DO).transpose(1, 0, 2))
        bh = b[h * DO : (h + 1) * DO].astype(ml_dtypes.bfloat16)
        bl = (
            b[h * DO : (h + 1) * DO] - bh.astype(np.float32)
        ).astype(ml_dtypes.bfloat16)
        biasr = np.ascontiguousarray(np.stack([bh, bl], axis=0))
        in_maps.append(
            {
                "x1T": _prep_xT(_bf16(input1[bb])),
                "x2T": _prep_xT(_bf16(input2[bb])),
                "w1r": w1r,
                "w2r": w2r,
                "eyer": eye,
                "biasr": biasr,
            }
        )
    return in_maps


def kernel(input1, input2, W, b):
    from concourse import bass_utils

    suppress_trace = False
    if os.environ.get("BASS_TRACE"):
        try:
            from antenv.axon_hooks import get_axon_ntff_profile_hook  # noqa: F401
        except Exception:
            suppress_trace = True
    prev = os.environ.get("BASS_NEVER_TRACE")
    if suppress_trace:
        os.environ["BASS_NEVER_TRACE"] = "1"
    try:
        nc = _get_module()
        in_maps = _make_in_maps(input1, input2, W, b)
        res = bass_utils.run_bass_kernel_spmd(
            nc, in_maps, core_ids=list(range(NCORES))
        )
    finally:
        if suppress_trace:
            if prev is None:
                os.environ.pop("BASS_NEVER_TRACE", None)
            else:
                os.environ["BASS_NEVER_TRACE"] = prev
    out = np.empty((4, NI, NJ, 2 * DO), dtype=np.float32)
    for c in range(NCORES):
        bb, h = divmod(c, 2)
        out[bb, :, :, h * DO : (h + 1) * DO] = np.asarray(
            res.results[c]["out"]
        ).astype(np.float32)
    return out


# revision 10
# speedup vs baseline: 1.7105x; 1.3824x over previous
"""Trainium2 Bass kernel for nn_CatEncoderCross.

Computes out[b,i,j,:] = input1[b,i,:] @ W[:768] + input2[b,j,:] @ W[768:] + bias
for shapes input1/input2 [4,128,768], W [1536,768], b [768],
output [4,128,128,768] f32 (~201 MB).

Sharding (batch, dout-half): core c handles batch c//2 and output columns
[384*(c%2), 384*(c%2)+384).

Per core, with j on partitions and 4 i-rows per group (one PSUM bank each):
  p1 = x1 @ W1[:, half] + bias   [128, 384]  (PE, single-plane bf16)
  p2 = x2 @ W2[:, half]          [128, 384]  (PE)
  group g (rows i0..i0+3):
    banks 0,1 (psA): psum = sel_i @ p1h          -> DVE adds p2, writes f16
    banks 2,3 (psE): psum = eye @ p2 + sel_i @ p1h -> Act copies psum to f16
  DMA [128, 4, 384] f16 tile -> HBM.

sel_i is a host-built selector matrix (row i%32 of a 32x128 all-ones-row
stack) that broadcasts p1 row i across all partitions straight from p1h —
no SBUF->SBUF flatten DMA.  The identity ("eye") matmul accumulates p2
into PSUM on the PE so the Activation engine (which cannot add vectors)
participates as a pure psum->f16 converter.  Output is written fp16 (host
upcasts); inputs are single-plane bf16.  Rel err ~4e-3 vs the f32
reference (gate 2e-2).
"""

import os
import numpy as np

P = 128
DO = 384  # output columns per core (dout/2)
KO = 6  # K chunks of 128 in d1 (=d2)
NI = 128  # n1 rows per core (full batch)
NJ = 128  # n2
NCORES = 8
FUSE = 4  # output rows per psum group / DMA

OUT_BUFS = int(os.environ.get("KERNEL_OUT_BUFS", "6"))
PSUM_BUFS = int(os.environ.get("KERNEL_PSUM_BUFS", "4"))
WARM_MMS = int(os.environ.get("KERNEL_WARM_MMS", "30"))
# perf-probe only: comma list of stages to skip (never set in real runs)
SKIP = set(s for s in os.environ.get("KERNEL_SKIP", "").split(",") if s)

_cache = {}


def _build_module():
    import concourse.bacc as bacc
    import concourse.mybir as mybir
    import concourse.tile as tile

    F32 = mybir.dt.float32
    F16 = mybir.dt.float16
    BF16 = mybir.dt.bfloat16

    nc = bacc.Bacc("TRN2", target_bir_lowering=False, debug=False)

    x1T_d = nc.dram_tensor("x1T", [P, KO, NI], BF16, kind="ExternalInput")
    x2T_d = nc.dram_tensor("x2T", [P, KO, NJ], BF16, kind="ExternalInput")
    w1_d = nc.dram_tensor("w1r", [P, KO, DO], BF16, kind="ExternalInput")
    w2_d = nc.dram_tensor("w2r", [P, KO, DO], BF16, kind="ExternalInput")
    sel_d = nc.dram_tensor("selr", [P, 32, P], BF16, kind="ExternalInput")
    eye_d = nc.dram_tensor("eyer", [P, P], BF16, kind="ExternalInput")
    bias_d = nc.dram_tensor("biasr", [2, DO], BF16, kind="ExternalInput")
    out_d = nc.dram_tensor("out", [NI, NJ, DO], F16, kind="ExternalOutput")
    out_ap = out_d.ap()

    with tile.TileContext(nc) as tc:
        with (
            tc.tile_pool(name="const", bufs=1) as cpool,
            tc.tile_pool(name="psum", bufs=PSUM_BUFS, space="PSUM") as pspool,
            tc.tile_pool(name="outp", bufs=OUT_BUFS) as opool,
        ):
            x1T_sb = cpool.tile([P, KO, NI], BF16, tag="x1T")
            x2T_sb = cpool.tile([P, KO, NJ], BF16, tag="x2T")
            w1_sb = cpool.tile([P, KO, DO], BF16, tag="w1")
            w2_sb = cpool.tile([P, KO, DO], BF16, tag="w2")
            sel_sb = cpool.tile([P, 32, P], BF16, tag="sel")
            eye_sb = cpool.tile([P, P], BF16, tag="eye")
            bias_sb = cpool.tile([2, DO], BF16, tag="bias")
            ones_bf = cpool.tile([2, P], BF16, tag="ones_bf")
            p1h_sb = cpool.tile([NI, DO], BF16, tag="p1h")
            p2bf_sb = cpool.tile([P, DO], BF16, tag="p2bf")

            nc.vector.memset(ones_bf[:], 1.0)

            if WARM_MMS:
                warm_ps = pspool.tile([P, 1024], F32, tag="ps", name="warm_ps")
                for wi in range(WARM_MMS):
                    nc.tensor.matmul(
                        warm_ps[:, 0:P],
                        ones_bf[0:2, :],
                        ones_bf[0:2, :],
                        start=True,
                        stop=True,
                    )

            # --- input DMAs (issue order = SP queue order) ---
            nc.sync.dma_start(out=x1T_sb[:], in_=x1T_d.ap())
            nc.sync.dma_start(out=w1_sb[:], in_=w1_d.ap())
            nc.sync.dma_start(out=x2T_sb[:], in_=x2T_d.ap())
            h = KO // 2
            nc.sync.dma_start(out=w2_sb[:, 0:h], in_=w2_d.ap()[:, 0:h])
            nc.sync.dma_start(out=w2_sb[:, h:KO], in_=w2_d.ap()[:, h:KO])
            nc.sync.dma_start(out=sel_sb[:], in_=sel_d.ap())
            nc.sync.dma_start(out=bias_sb[:], in_=bias_d.ap())
            nc.sync.dma_start(out=eye_sb[:], in_=eye_d.ap())

            def sel_mm(dst, i, start, stop):
                g32 = (i // 32) * 32
                nc.tensor.matmul(
                    dst,
                    sel_sb[g32 : g32 + 32, i % 32, :],
                    p1h_sb[g32 : g32 + 32, :],
                    start=start,
                    stop=stop,
                    tile_position=(g32, 0),
                )

            if "proj" not in SKIP:
                # --- p1 = x1 @ W1h + bias (bias-mm last: its DMA lands
                # late but before chunk mms finish; keeps p2 chain short) ---
                p1_ps = pspool.tile([P, 1024], F32, tag="ps", name="p1_ps")
                for o in range(KO):
                    nc.tensor.matmul(
                        p1_ps[:, 0:DO],
                        x1T_sb[:, o, :],
                        w1_sb[:, o, :],
                        start=(o == 0),
                        stop=False,
                    )
                nc.tensor.matmul(
                    p1_ps[:, 0:DO],
                    ones_bf[0:2, :],
                    bias_sb[:],
                    start=False,
                    stop=True,
                )
                nc.vector.tensor_copy(out=p1h_sb[:], in_=p1_ps[:, 0:DO])

                # --- p2 = x2 @ W2h ---
                p2_ps = pspool.tile([P, 1024], F32, tag="ps", name="p2_ps")
                for o in range(KO):
                    nc.tensor.matmul(
                        p2_ps[:, 0:DO],
                        x2T_sb[:, o, :],
                        w2_sb[:, o, :],
                        start=(o == 0),
                        stop=(o == KO - 1),
                    )
                nc.vector.tensor_copy(out=p2bf_sb[:], in_=p2_ps[:, 0:DO])

            # --- main loop: 32 groups of 4 rows ---
            # Two 2-bank psum tiles per group: psA (DVE-consumed) and psE
            # (Act-consumed) recycle independently.
            for grp in range(NI // FUSE):
                i0 = FUSE * grp
                ob = opool.tile([P, FUSE, DO], F16, tag="ob", name=f"ob{grp}")
                psA = pspool.tile([P, 1024], F32, tag="ps", name=f"psA{grp}")
                psE = pspool.tile([P, 1024], F32, tag="ps", name=f"psE{grp}")
                if "mm1" not in SKIP:
                    for m in (0, 1):
                        sel_mm(
                            psA[:, 512 * m : 512 * m + DO],
                            i0 + m,
                            start=True,
                            stop=True,
                        )
                    for m in (2, 3):
                        dst = psE[:, 512 * (m - 2) : 512 * (m - 2) + DO]
                        nc.tensor.matmul(
                            dst, eye_sb[:], p2bf_sb[:], start=True, stop=False
                        )
                        sel_mm(dst, i0 + m, start=False, stop=True)
                psA_v = psA.rearrange("p (i x) -> p i x", i=2)[:, :, 0:DO]
                psE_v = psE.rearrange("p (i x) -> p i x", i=2)[:, :, 0:DO]
                if "add" not in SKIP:
                    p2_b = p2bf_sb[:, None, :].to_broadcast((P, 2, DO))
                    nc.vector.tensor_add(
                        out=ob[:, 0:2, :], in0=psA_v, in1=p2_b
                    )
                    nc.scalar.copy(out=ob[:, 2:4, :], in_=psE_v)
                if "dmaout" not in SKIP:
                    if grp == 0:
                        # split first tile: the write stream starts as soon
                        # as the DVE half is ready
                        nc.sync.dma_start(
                            out=out_ap[i0 : i0 + 2].rearrange(
                                "i j d -> j i d"
                            ),
                            in_=ob[:, 0:2, :],
                        )
                        nc.sync.dma_start(
                            out=out_ap[i0 + 2 : i0 + 4].rearrange(
                                "i j d -> j i d"
                            ),
                            in_=ob[:, 2:4, :],
                        )
                    else:
                        dst = out_ap[i0 : i0 + FUSE]  # [FUSE, NJ, DO]
                        nc.sync.dma_start(
                            out=dst.rearrange("i j d -> j i d"), in_=ob[:]
                        )

    nc.compile()
    return nc


def _get_module():
    key = (OUT_BUFS, PSUM_BUFS, WARM_MMS, tuple(sorted(SKIP)))
    if key not in _cache:
        _cache[key] = _build_module()
    return _cache[key]


def _bf16(x):
    import ml_dtypes

    return np.asarray(x, dtype=np.float32).astype(ml_dtypes.bfloat16)


def _prep_xT(x):
    """[128, 768] -> [128, KO, 128] transposed chunk layout (bf16)."""
    return np.ascontiguousarray(x.T.reshape(KO, P, P).transpose(1, 0, 2))


def _make_in_maps(input1, input2, W, b):
    import ml_dtypes

    input1 = np.asarray(input1, dtype=np.float32)
    input2 = np.asarray(input2, dtype=np.float32)
    W = np.asarray(W, dtype=np.float32)
    b = np.asarray(b, dtype=np.float32)

    eye = np.eye(P, dtype=ml_dtypes.bfloat16)
    sel = np.zeros((P, 32, P), dtype=ml_dtypes.bfloat16)
    sel[np.arange(P), np.arange(P) % 32, :] = 1.0
    in_maps = []
    for c in range(NCORES):
        bb, h = divmod(c, 2)
        W1 = _bf16(W[:768, h * DO : (h + 1) * DO])
        W2 = _bf16(W[768:, h * DO : (h + 1) * DO])
        w1r = np.ascontiguousarray(W1.reshape(KO, P, DO).transpose(1, 0, 2))
        w2r = np.ascontiguousarray(W2.reshape(KO, P, DO).transpose(1, 0, 2))
        bh = b[h * DO : (h + 1) * DO].astype(ml_dtypes.bfloat16)
        bl = (
            b[h * DO : (h + 1) * DO] - bh.astype(np.float32)
        ).astype(ml_dtypes.bfloat16)
        biasr = np.ascontiguousarray(np.stack([bh, bl], axis=0))
        in_maps.append(
            {
                "x1T": _prep_xT(_bf16(input1[bb])),
                "x2T": _prep_xT(_bf16(input2[bb])),
                "w1r": w1r,
                "w2r": w2r,
                "selr": sel,
                "eyer": eye,
                "biasr": biasr,
            }
        )
    return in_maps


def kernel(input1, input2, W, b):
    from concourse import bass_utils

    suppress_trace = False
    if os.environ.get("BASS_TRACE"):
        try:
            from antenv.axon_hooks import get_axon_ntff_profile_hook  # noqa: F401
        except Exception:
            suppress_trace = True
    prev = os.environ.get("BASS_NEVER_TRACE")
    if suppress_trace:
        os.environ["BASS_NEVER_TRACE"] = "1"
    try:
        nc = _get_module()
        in_maps = _make_in_maps(input1, input2, W, b)
        res = bass_utils.run_bass_kernel_spmd(
            nc, in_maps, core_ids=list(range(NCORES))
        )
    finally:
        if suppress_trace:
            if prev is None:
                os.environ.pop("BASS_NEVER_TRACE", None)
            else:
                os.environ["BASS_NEVER_TRACE"] = prev
    out = np.empty((4, NI, NJ, 2 * DO), dtype=np.float32)
    for c in range(NCORES):
        bb, h = divmod(c, 2)
        out[bb, :, :, h * DO : (h + 1) * DO] = np.asarray(
            res.results[c]["out"]
        ).astype(np.float32)
    return out


# revision 16
# speedup vs baseline: 1.7995x; 1.0520x over previous
"""Trainium2 Bass kernel for nn_CatEncoderCross.

Computes out[b,i,j,:] = input1[b,i,:] @ W[:768] + input2[b,j,:] @ W[768:] + bias
for shapes input1/input2 [4,128,768], W [1536,768], b [768],
output [4,128,128,768] f32 (~201 MB).

Sharding (batch, dout-half): core c handles batch c//2 and output columns
[384*(c%2), 384*(c%2)+384).

Per core, with j on partitions and 4 i-rows per group (one PSUM bank each):
  p1 = x1 @ W1[:, half] + bias   [128, 384]  (PE, single-plane bf16)
  p2 = x2 @ W2[:, half]          [128, 384]  (PE)
  group g (rows i0..i0+3):
    banks 0,1 (psA): psum = sel_i @ p1h          -> DVE adds p2, writes f16
    banks 2,3 (psE): psum = eye @ p2 + sel_i @ p1h -> Act copies psum to f16
  DMA [128, 4, 384] f16 tile -> HBM.

sel_i is a host-built selector matrix (row i%32 of a 32x128 all-ones-row
stack) that broadcasts p1 row i across all partitions straight from p1h —
no SBUF->SBUF flatten DMA.  The identity ("eye") matmul accumulates p2
into PSUM on the PE so the Activation engine (which cannot add vectors)
participates as a pure psum->f16 converter.  Output is written fp16 (host
upcasts); inputs are single-plane bf16.  Rel err ~4e-3 vs the f32
reference (gate 2e-2).
"""

import os
import numpy as np

P = 128
DO = 384  # output columns per core (dout/2)
KO = 6  # K chunks of 128 in d1 (=d2)
NI = 128  # n1 rows per core (full batch)
NJ = 128  # n2
NCORES = 8
FUSE = 4  # output rows per psum group / DMA

OUT_BUFS = int(os.environ.get("KERNEL_OUT_BUFS", "6"))
PSUM_BUFS = int(os.environ.get("KERNEL_PSUM_BUFS", "4"))
WARM_MMS = int(os.environ.get("KERNEL_WARM_MMS", "36"))
# perf-probe only: comma list of stages to skip (never set in real runs)
SKIP = set(s for s in os.environ.get("KERNEL_SKIP", "").split(",") if s)

_cache = {}


def _build_module():
    import concourse.bacc as bacc
    import concourse.mybir as mybir
    import concourse.tile as tile

    F32 = mybir.dt.float32
    F16 = mybir.dt.float16
    BF16 = mybir.dt.bfloat16

    nc = bacc.Bacc("TRN2", target_bir_lowering=False, debug=False)

    # x1T/w1r carry an extra K=32 "chunk" (index KO) holding a ones row and
    # the bias row: the p1 projection's last matmul adds the bias with no
    # separate bias DMA on the critical path.
    x1T_d = nc.dram_tensor("x1T", [P, KO + 1, NI], BF16, kind="ExternalInput")
    x2T_d = nc.dram_tensor("x2T", [P, KO, NJ], BF16, kind="ExternalInput")
    w1_d = nc.dram_tensor("w1r", [P, KO + 1, DO], BF16, kind="ExternalInput")
    w2_d = nc.dram_tensor("w2r", [P, KO, DO], BF16, kind="ExternalInput")
    sel_d = nc.dram_tensor("selr", [P, 32, P], BF16, kind="ExternalInput")
    eye_d = nc.dram_tensor("eyer", [P, P], BF16, kind="ExternalInput")
    out_d = nc.dram_tensor("out", [NI, NJ, DO], F16, kind="ExternalOutput")
    out_ap = out_d.ap()

    with tile.TileContext(nc) as tc:
        with (
            tc.tile_pool(name="const", bufs=1) as cpool,
            tc.tile_pool(name="psum", bufs=PSUM_BUFS, space="PSUM") as pspool,
            tc.tile_pool(name="outp", bufs=OUT_BUFS) as opool,
        ):
            x1T_sb = cpool.tile([P, KO + 1, NI], BF16, tag="x1T")
            x2T_sb = cpool.tile([P, KO, NJ], BF16, tag="x2T")
            w1_sb = cpool.tile([P, KO + 1, DO], BF16, tag="w1")
            w2_sb = cpool.tile([P, KO, DO], BF16, tag="w2")
            sel_sb = cpool.tile([P, 32, P], BF16, tag="sel")
            eye_sb = cpool.tile([P, P], BF16, tag="eye")
            ones_bf = cpool.tile([2, P], BF16, tag="ones_bf")
            p1h_sb = cpool.tile([NI, DO], BF16, tag="p1h")
            p2bf_sb = cpool.tile([P, DO], BF16, tag="p2bf")

            nc.vector.memset(ones_bf[:], 1.0)

            if WARM_MMS:
                warm_ps = pspool.tile([P, 1024], F32, tag="ps", name="warm_ps")
                for wi in range(WARM_MMS):
                    nc.tensor.matmul(
                        warm_ps[:, 0:P],
                        ones_bf[0:2, :],
                        ones_bf[0:2, :],
                        start=True,
                        stop=True,
                    )

            # --- input DMAs (issue order = SP queue order).  sel (1 MB) is
            # split by variant range so group 0 only waits for the first
            # quarter; it streams during the otherwise-idle pre-output
            # window. ---
            nc.sync.dma_start(out=x1T_sb[:], in_=x1T_d.ap())
            nc.sync.dma_start(out=w1_sb[:], in_=w1_d.ap())
            nc.sync.dma_start(out=x2T_sb[:], in_=x2T_d.ap())
            h = KO // 2
            nc.sync.dma_start(out=w2_sb[:, 0:h], in_=w2_d.ap()[:, 0:h])
            nc.sync.dma_start(out=w2_sb[:, h:KO], in_=w2_d.ap()[:, h:KO])
            nc.sync.dma_start(out=eye_sb[:], in_=eye_d.ap())
            for v0 in range(0, 32, 8):
                nc.sync.dma_start(
                    out=sel_sb[:, v0 : v0 + 8], in_=sel_d.ap()[:, v0 : v0 + 8]
                )

            def sel_mm(dst, i, start, stop):
                g32 = (i // 32) * 32
                nc.tensor.matmul(
                    dst,
                    sel_sb[g32 : g32 + 32, i % 32, :],
                    p1h_sb[g32 : g32 + 32, :],
                    start=start,
                    stop=stop,
                    tile_position=(g32, 0),
                )

            if "proj" not in SKIP:
                # --- p1 = x1 @ W1h + bias (chunk KO is the bias chunk) ---
                p1_ps = pspool.tile([P, 1024], F32, tag="ps", name="p1_ps")
                for o in range(KO):
                    nc.tensor.matmul(
                        p1_ps[:, 0:DO],
                        x1T_sb[:, o, :],
                        w1_sb[:, o, :],
                        start=(o == 0),
                        stop=False,
                    )
                nc.tensor.matmul(
                    p1_ps[:, 0:DO],
                    x1T_sb[0:32, KO, :],
                    w1_sb[0:32, KO, :],
                    start=False,
                    stop=True,
                )
                nc.vector.tensor_copy(out=p1h_sb[:], in_=p1_ps[:, 0:DO])

                # --- p2 = x2 @ W2h ---
                p2_ps = pspool.tile([P, 1024], F32, tag="ps", name="p2_ps")
                for o in range(KO):
                    nc.tensor.matmul(
                        p2_ps[:, 0:DO],
                        x2T_sb[:, o, :],
                        w2_sb[:, o, :],
                        start=(o == 0),
                        stop=(o == KO - 1),
                    )
                nc.vector.tensor_copy(out=p2bf_sb[:], in_=p2_ps[:, 0:DO])

            # --- main loop: 32 groups of 4 rows ---
            # Two 2-bank psum tiles per group: psA (DVE-consumed) and psE
            # (Act-consumed) recycle independently.
            for grp in range(NI // FUSE):
                i0 = FUSE * grp
                ob = opool.tile([P, FUSE, DO], F16, tag="ob", name=f"ob{grp}")
                psA = pspool.tile([P, 1024], F32, tag="ps", name=f"psA{grp}")
                psE = pspool.tile([P, 1024], F32, tag="ps", name=f"psE{grp}")
                if "mm1" not in SKIP:
                    for m in (0, 1):
                        sel_mm(
                            psA[:, 512 * m : 512 * m + DO],
                            i0 + m,
                            start=True,
                            stop=True,
                        )
                    for m in (2, 3):
                        dst = psE[:, 512 * (m - 2) : 512 * (m - 2) + DO]
                        nc.tensor.matmul(
                            dst, eye_sb[:], p2bf_sb[:], start=True, stop=False
                        )
                        sel_mm(dst, i0 + m, start=False, stop=True)
                psA_v = psA.rearrange("p (i x) -> p i x", i=2)[:, :, 0:DO]
                psE_v = psE.rearrange("p (i x) -> p i x", i=2)[:, :, 0:DO]
                if "add" not in SKIP:
                    p2_b = p2bf_sb[:, None, :].to_broadcast((P, 2, DO))
                    nc.vector.tensor_add(
                        out=ob[:, 0:2, :], in0=psA_v, in1=p2_b
                    )
                    nc.scalar.copy(out=ob[:, 2:4, :], in_=psE_v)
                if "dmaout" not in SKIP:
                    if grp == 0:
                        # split first tile: the write stream starts as soon
                        # as the DVE half is ready
                        nc.sync.dma_start(
                            out=out_ap[i0 : i0 + 2].rearrange(
                                "i j d -> j i d"
                            ),
                            in_=ob[:, 0:2, :],
                        )
                        nc.sync.dma_start(
                            out=out_ap[i0 + 2 : i0 + 4].rearrange(
                                "i j d -> j i d"
                            ),
                            in_=ob[:, 2:4, :],
                        )
                    else:
                        dst = out_ap[i0 : i0 + FUSE]  # [FUSE, NJ, DO]
                        nc.sync.dma_start(
                            out=dst.rearrange("i j d -> j i d"), in_=ob[:]
                        )

    nc.compile()
    return nc


def _get_module():
    key = (OUT_BUFS, PSUM_BUFS, WARM_MMS, tuple(sorted(SKIP)))
    if key not in _cache:
        _cache[key] = _build_module()
    return _cache[key]


def _bf16(x):
    import ml_dtypes

    return np.asarray(x, dtype=np.float32).astype(ml_dtypes.bfloat16)


def _prep_xT(x):
    """[128, 768] -> [128, KO, 128] transposed chunk layout (bf16)."""
    return np.ascontiguousarray(x.T.reshape(KO, P, P).transpose(1, 0, 2))


def _make_in_maps(input1, input2, W, b):
    import ml_dtypes

    input1 = np.asarray(input1, dtype=np.float32)
    input2 = np.asarray(input2, dtype=np.float32)
    W = np.asarray(W, dtype=np.float32)
    b = np.asarray(b, dtype=np.float32)

    eye = np.eye(P, dtype=ml_dtypes.bfloat16)
    sel = np.zeros((P, 32, P), dtype=ml_dtypes.bfloat16)
    sel[np.arange(P), np.arange(P) % 32, :] = 1.0
    # bias chunk: x1T[:, KO] is a ones row on partition 0; w1r[:, KO] is b.
    x1c7 = np.zeros((P, 1, NI), dtype=ml_dtypes.bfloat16)
    x1c7[0, 0, :] = 1.0
    in_maps = []
    for c in range(NCORES):
        bb, h = divmod(c, 2)
        W1 = _bf16(W[:768, h * DO : (h + 1) * DO])
        W2 = _bf16(W[768:, h * DO : (h + 1) * DO])
        w1r = W1.reshape(KO, P, DO).transpose(1, 0, 2)
        w2r = np.ascontiguousarray(W2.reshape(KO, P, DO).transpose(1, 0, 2))
        w1c7 = np.zeros((P, 1, DO), dtype=ml_dtypes.bfloat16)
        w1c7[0, 0, :] = _bf16(b[h * DO : (h + 1) * DO])
        in_maps.append(
            {
                "x1T": np.ascontiguousarray(
                    np.concatenate([_prep_xT(_bf16(input1[bb])), x1c7], 1)
                ),
                "x2T": _prep_xT(_bf16(input2[bb])),
                "w1r": np.ascontiguousarray(
                    np.concatenate([w1r, w1c7], axis=1)
                ),
                "w2r": w2r,
                "selr": sel,
                "eyer": eye,
            }
        )
    return in_maps


def kernel(input1, input2, W, b):
    from concourse import bass_utils

    suppress_trace = False
    if os.environ.get("BASS_TRACE"):
        try:
            from antenv.axon_hooks import get_axon_ntff_profile_hook  # noqa: F401
        except Exception:
            suppress_trace = True
    prev = os.environ.get("BASS_NEVER_TRACE")
    if suppress_trace:
        os.environ["BASS_NEVER_TRACE"] = "1"
    try:
        nc = _get_module()
        in_maps = _make_in_maps(input1, input2, W, b)
        res = bass_utils.run_bass_kernel_spmd(
            nc, in_maps, core_ids=list(range(NCORES))
        )
    finally:
        if suppress_trace:
            if prev is None:
                os.environ.pop("BASS_NEVER_TRACE", None)
            else:
                os.environ["BASS_NEVER_TRACE"] = prev
    out = np.empty((4, NI, NJ, 2 * DO), dtype=np.float32)
    for c in range(NCORES):
        bb, h = divmod(c, 2)
        out[bb, :, :, h * DO : (h + 1) * DO] = np.asarray(
            res.results[c]["out"]
        ).astype(np.float32)
    return out


# revision 35
# speedup vs baseline: 1.8200x; 1.0114x over previous
"""Trainium2 Bass kernel for nn_CatEncoderCross.

Computes out[b,i,j,:] = input1[b,i,:] @ W[:768] + input2[b,j,:] @ W[768:] + bias
for shapes input1/input2 [4,128,768], W [1536,768], b [768],
output [4,128,128,768] f32 (~201 MB).

Sharding (batch, dout-half): core c handles batch c//2 and output columns
[384*(c%2), 384*(c%2)+384).

Per core, with j on partitions and 4 i-rows per group (one PSUM bank each):
  p1 = x1 @ W1[:, half] + bias   [128, 384]  (PE, single-plane bf16)
  p2 = x2 @ W2[:, half]          [128, 384]  (PE)
  group g (rows i0..i0+3):
    banks 0,1 (psA): psum = sel_i @ p1h          -> DVE adds p2, writes f16
    banks 2,3 (psE): psum = eye @ p2 + sel_i @ p1h -> Act copies psum to f16
  DMA [128, 4, 384] f16 tile -> HBM.

sel_i is a host-built selector matrix (row i%32 of a 32x128 all-ones-row
stack) that broadcasts p1 row i across all partitions straight from p1h —
no SBUF->SBUF flatten DMA.  The identity ("eye") matmul accumulates p2
into PSUM on the PE so the Activation engine (which cannot add vectors)
participates as a pure psum->f16 converter.  Output is written fp16 (host
upcasts); inputs are single-plane bf16.  Rel err ~4e-3 vs the f32
reference (gate 2e-2).
"""

import os
import numpy as np

P = 128
DO = 384  # output columns per core (dout/2)
KO = 6  # K chunks of 128 in d1 (=d2)
NI = 128  # n1 rows per core (full batch)
NJ = 128  # n2
NCORES = 8
FUSE = 4  # output rows per psum group / DMA

OUT_BUFS = int(os.environ.get("KERNEL_OUT_BUFS", "8"))
PSUM_BUFS = int(os.environ.get("KERNEL_PSUM_BUFS", "4"))
WARM_MMS = int(os.environ.get("KERNEL_WARM_MMS", "36"))
# perf-probe only: comma list of stages to skip (never set in real runs)
SKIP = set(s for s in os.environ.get("KERNEL_SKIP", "").split(",") if s)

_cache = {}


def _build_module():
    import concourse.bacc as bacc
    import concourse.mybir as mybir
    import concourse.tile as tile

    F32 = mybir.dt.float32
    F16 = mybir.dt.float16
    BF16 = mybir.dt.bfloat16

    nc = bacc.Bacc("TRN2", target_bir_lowering=False, debug=False)

    # x1T/w1r carry an extra K=32 "chunk" (index KO) holding a ones row and
    # the bias row: the p1 projection's last matmul adds the bias with no
    # separate bias DMA on the critical path.
    x1T_d = nc.dram_tensor("x1T", [P, KO + 1, NI], BF16, kind="ExternalInput")
    x2T_d = nc.dram_tensor("x2T", [P, KO, NJ], BF16, kind="ExternalInput")
    w1_d = nc.dram_tensor("w1r", [P, KO + 1, DO], BF16, kind="ExternalInput")
    w2_d = nc.dram_tensor("w2r", [P, KO, DO], BF16, kind="ExternalInput")
    sel_d = nc.dram_tensor("selr", [P, 32, P], BF16, kind="ExternalInput")
    eye_d = nc.dram_tensor("eyer", [P, P], BF16, kind="ExternalInput")
    out_d = nc.dram_tensor("out", [NI, NJ, DO], F16, kind="ExternalOutput")
    out_ap = out_d.ap()

    with tile.TileContext(nc) as tc:
        with (
            tc.tile_pool(name="const", bufs=1) as cpool,
            tc.tile_pool(name="psum", bufs=PSUM_BUFS, space="PSUM") as pspool,
            tc.tile_pool(name="outp", bufs=OUT_BUFS) as opool,
        ):
            x1T_sb = cpool.tile([P, KO + 1, NI], BF16, tag="x1T")
            x2T_sb = cpool.tile([P, KO, NJ], BF16, tag="x2T")
            w1_sb = cpool.tile([P, KO + 1, DO], BF16, tag="w1")
            w2_sb = cpool.tile([P, KO, DO], BF16, tag="w2")
            sel_sb = cpool.tile([P, 32, P], BF16, tag="sel")
            eye_sb = cpool.tile([P, P], BF16, tag="eye")
            ones_bf = cpool.tile([2, P], BF16, tag="ones_bf")
            p1h_sb = cpool.tile([NI, DO], BF16, tag="p1h")
            p2bf_sb = cpool.tile([P, DO], BF16, tag="p2bf")

            nc.vector.memset(ones_bf[:], 1.0)

            if WARM_MMS:
                warm_ps = pspool.tile([P, 1024], F32, tag="ps", name="warm_ps")
                for wi in range(WARM_MMS):
                    nc.tensor.matmul(
                        warm_ps[:, 0:P],
                        ones_bf[0:2, :],
                        ones_bf[0:2, :],
                        start=True,
                        stop=True,
                    )

            # --- input DMAs (issue order = SP queue order).  sel (1 MB) is
            # split by variant range so group 0 only waits for the first
            # quarter; it streams during the otherwise-idle pre-output
            # window. ---
            nc.sync.dma_start(out=x1T_sb[:], in_=x1T_d.ap())
            nc.sync.dma_start(out=w1_sb[:], in_=w1_d.ap())
            nc.sync.dma_start(out=x2T_sb[:], in_=x2T_d.ap())
            # w2 in 3/2/1 chunk pieces: the final piece's DMA-completion
            # semaphore then covers only one trailing matmul
            for o0, o1 in ((0, 3), (3, 5), (5, 6)):
                nc.sync.dma_start(out=w2_sb[:, o0:o1], in_=w2_d.ap()[:, o0:o1])
            nc.sync.dma_start(out=sel_sb[:, 0:4], in_=sel_d.ap()[:, 0:4])
            nc.sync.dma_start(out=eye_sb[:], in_=eye_d.ap())
            for v0, v1 in ((4, 12), (12, 22), (22, 32)):
                nc.sync.dma_start(
                    out=sel_sb[:, v0:v1], in_=sel_d.ap()[:, v0:v1]
                )

            def sel_mm(dst, i, start, stop):
                g32 = (i // 32) * 32
                nc.tensor.matmul(
                    dst,
                    sel_sb[g32 : g32 + 32, i % 32, :],
                    p1h_sb[g32 : g32 + 32, :],
                    start=start,
                    stop=stop,
                    tile_position=(g32, 0),
                )

            if "proj" not in SKIP:
                # --- p1 = x1 @ W1h + bias (chunk KO is the bias chunk) ---
                p1_ps = pspool.tile([P, 1024], F32, tag="ps", name="p1_ps")
                for o in range(KO):
                    nc.tensor.matmul(
                        p1_ps[:, 0:DO],
                        x1T_sb[:, o, :],
                        w1_sb[:, o, :],
                        start=(o == 0),
                        stop=False,
                    )
                nc.tensor.matmul(
                    p1_ps[:, 0:DO],
                    x1T_sb[0:32, KO, :],
                    w1_sb[0:32, KO, :],
                    start=False,
                    stop=True,
                )
                nc.vector.tensor_copy(out=p1h_sb[:], in_=p1_ps[:, 0:DO])

                # --- p2 = x2 @ W2h; the (otherwise idle) Act engine makes
                # the bf16 copy used by both the eye-matmuls and the DVE
                # adds ---
                p2_ps = pspool.tile([P, 1024], F32, tag="ps", name="p2_ps")
                for o in range(KO):
                    nc.tensor.matmul(
                        p2_ps[:, 0:DO],
                        x2T_sb[:, o, :],
                        w2_sb[:, o, :],
                        start=(o == 0),
                        stop=(o == KO - 1),
                    )
                nc.scalar.mul(p2bf_sb[:], p2_ps[:, 0:DO], 1.0)

            # --- main loop: 32 groups of 4 rows ---
            # Two 2-bank psum tiles per group: psA (DVE-consumed) and psE
            # (Act-consumed) recycle independently.
            for grp in range(NI // FUSE):
                i0 = FUSE * grp
                ob = opool.tile([P, FUSE, DO], F16, tag="ob", name=f"ob{grp}")
                psA = pspool.tile([P, 1024], F32, tag="ps", name=f"psA{grp}")
                psE = pspool.tile([P, 1024], F32, tag="ps", name=f"psE{grp}")
                if "mm1" not in SKIP:
                    for m in (0, 1):
                        sel_mm(
                            psA[:, 512 * m : 512 * m + DO],
                            i0 + m,
                            start=True,
                            stop=True,
                        )
                    for m in (2, 3):
                        dst = psE[:, 512 * (m - 2) : 512 * (m - 2) + DO]
                        nc.tensor.matmul(
                            dst, eye_sb[:], p2bf_sb[:], start=True, stop=False
                        )
                        sel_mm(dst, i0 + m, start=False, stop=True)
                psA_v = psA.rearrange("p (i x) -> p i x", i=2)[:, :, 0:DO]
                psE_v = psE.rearrange("p (i x) -> p i x", i=2)[:, :, 0:DO]
                p2_b = p2bf_sb[:, None, :].to_broadcast((P, 2, DO))
                if grp == 0:
                    # fully split first group: each row ships the moment its
                    # add lands, so the write stream starts earliest
                    if "add" not in SKIP:
                        for m in (0, 1):
                            nc.vector.tensor_add(
                                out=ob[:, m : m + 1, :],
                                in0=psA_v[:, m : m + 1, :],
                                in1=p2bf_sb[:, None, :].to_broadcast(
                                    (P, 1, DO)
                                ),
                            )
                            if "dmaout" not in SKIP:
                                nc.sync.dma_start(
                                    out=out_ap[m : m + 1].rearrange(
                                        "i j d -> j i d"
                                    ),
                                    in_=ob[:, m : m + 1, :],
                                )
                        nc.scalar.copy(out=ob[:, 2:4, :], in_=psE_v)
                        if "dmaout" not in SKIP:
                            nc.sync.dma_start(
                                out=out_ap[2:4].rearrange("i j d -> j i d"),
                                in_=ob[:, 2:4, :],
                            )
                    continue
                if "add" not in SKIP:
                    nc.vector.tensor_add(
                        out=ob[:, 0:2, :], in0=psA_v, in1=p2_b
                    )
                    nc.scalar.copy(out=ob[:, 2:4, :], in_=psE_v)
                if "dmaout" not in SKIP:
                    dst = out_ap[i0 : i0 + FUSE]  # [FUSE, NJ, DO]
                    nc.sync.dma_start(
                        out=dst.rearrange("i j d -> j i d"), in_=ob[:]
                    )

    nc.compile()
    return nc


def _get_module():
    key = (OUT_BUFS, PSUM_BUFS, WARM_MMS, tuple(sorted(SKIP)))
    if key not in _cache:
        _cache[key] = _build_module()
    return _cache[key]


def _bf16(x):
    import ml_dtypes

    return np.asarray(x, dtype=np.float32).astype(ml_dtypes.bfloat16)


def _prep_xT(x):
    """[128, 768] -> [128, KO, 128] transposed chunk layout (bf16)."""
    return np.ascontiguousarray(x.T.reshape(KO, P, P).transpose(1, 0, 2))


def _make_in_maps(input1, input2, W, b):
    import ml_dtypes

    input1 = np.asarray(input1, dtype=np.float32)
    input2 = np.asarray(input2, dtype=np.float32)
    W = np.asarray(W, dtype=np.float32)
    b = np.asarray(b, dtype=np.float32)

    eye = np.eye(P, dtype=ml_dtypes.bfloat16)
    sel = np.zeros((P, 32, P), dtype=ml_dtypes.bfloat16)
    sel[np.arange(P), np.arange(P) % 32, :] = 1.0
    # bias chunk: x1T[:, KO] is a ones row on partition 0; w1r[:, KO] is b.
    x1c7 = np.zeros((P, 1, NI), dtype=ml_dtypes.bfloat16)
    x1c7[0, 0, :] = 1.0
    in_maps = []
    for c in range(NCORES):
        bb, h = divmod(c, 2)
        W1 = _bf16(W[:768, h * DO : (h + 1) * DO])
        W2 = _bf16(W[768:, h * DO : (h + 1) * DO])
        w1r = W1.reshape(KO, P, DO).transpose(1, 0, 2)
        w2r = np.ascontiguousarray(W2.reshape(KO, P, DO).transpose(1, 0, 2))
        w1c7 = np.zeros((P, 1, DO), dtype=ml_dtypes.bfloat16)
        w1c7[0, 0, :] = _bf16(b[h * DO : (h + 1) * DO])
        in_maps.append(
            {
                "x1T": np.ascontiguousarray(
                    np.concatenate([_prep_xT(_bf16(input1[bb])), x1c7], 1)
                ),
                "x2T": _prep_xT(_bf16(input2[bb])),
                "w1r": np.ascontiguousarray(
                    np.concatenate([w1r, w1c7], axis=1)
                ),
                "w2r": w2r,
                "selr": sel,
                "eyer": eye,
            }
        )
    return in_maps


def kernel(input1, input2, W, b):
    from concourse import bass_utils

    suppress_trace = False
    if os.environ.get("BASS_TRACE"):
        try:
            from antenv.axon_hooks import get_axon_ntff_profile_hook  # noqa: F401
        except Exception:
            suppress_trace = True
    prev = os.environ.get("BASS_NEVER_TRACE")
    if suppress_trace:
        os.environ["BASS_NEVER_TRACE"] = "1"
    try:
        nc = _get_module()
        in_maps = _make_in_maps(input1, input2, W, b)
        res = bass_utils.run_bass_kernel_spmd(
            nc, in_maps, core_ids=list(range(NCORES))
        )
    finally:
        if suppress_trace:
            if prev is None:
                os.environ.pop("BASS_NEVER_TRACE", None)
            else:
                os.environ["BASS_NEVER_TRACE"] = prev
    out = np.empty((4, NI, NJ, 2 * DO), dtype=np.float32)
    for c in range(NCORES):
        bb, h = divmod(c, 2)
        out[bb, :, :, h * DO : (h + 1) * DO] = np.asarray(
            res.results[c]["out"]
        ).astype(np.float32)
    return out
